# revision 2
# baseline (speedup 1.0000x reference)
"""Trainium2 Bass kernel for nn_AttentionLayer. v7

Changes from v4:
  - fp8 (e4m3) DoubleRow matmuls for QKV projection, PV, and den:
    0.5 cycles/row on the PE (2x bf16).  Scores stay bf16 (K=64
    row-packed pairs; DoubleRow cannot help a 64-deep contraction).
  - score PSUM tiles are [128, 2, 512] pairs; ONE exp per pair
    (halves ACT per-instruction overhead) writing fp8 directly in the
    [K, 2, N] layout DoubleRow consumes.
  - den computed per-pair via a DoubleRow ones-matmul (one [1,512]
    row, no col-group halves to add).
  - reciprocal_approx_fast (custom DVE op, ~5x faster than
    reciprocal) for the softmax denominator.
  - prologue chunks interleaved with j-chunk-0 attention pairs so the
    ACT exp stream (the bottleneck engine) starts ~3us in instead of
    after the full prologue.  relu runs on ACT, qkv bias-copies and
    vtg quantize on DVE, to balance the two engines in that phase.
"""

import numpy as np

import bass_rust
import concourse.bass as bass
import concourse.tile as tile
from concourse import mybir
from concourse.bass_utils import run_bass_kernel_spmd

N_CORES = 8
C = 256
M = 64
HW = 4096
JC = 512
N_JC = HW // JC          # 8 j-chunks
N_IT = HW // 128         # 32 i-tiles
N_PAIR = N_IT // 2       # 16 i-tile pairs per j-chunk

F32 = mybir.dt.float32
BF16 = mybir.dt.bfloat16
F8 = mybir.dt.float8e4
DR = mybir.MatmulPerfMode.DoubleRow
AF = mybir.ActivationFunctionType


def _install_tile_drain_fix():
    def _drain_and_barrier(self, tick_clock, wait_clock):
        from concourse.tile import ScopedClock

        nc = self.nc
        probe = nc.sync.nop()
        wait_clock.add_sem_waits(
            probe.ins, ScopedClock({None: tick_clock.global_clock})
        )
        si = probe.ins.sync_info
        waits = list(si.on_wait) if si is not None else []
        probe.ins.sync_info = bass_rust.SyncInfo(on_wait=waits[:1], on_update=[])
        for w in waits[1:]:
            n = nc.sync.nop()
            n.ins.sync_info = bass_rust.SyncInfo(on_wait=[w], on_update=[])
        nc.sync.drain()
        nc.all_engine_barrier()
        assert self.sems is not None
        popped = nc._tile_sem_poison_stack.pop()
        assert popped is self._sem_poison
        nc.clear_and_free_semaphores(list(self.sems.allocated().values()))
        nc.all_engine_barrier()

    tile.TileContext._drain_and_barrier = _drain_and_barrier


def _split_multi_waits(nc):
    """walrus in this toolchain encodes at most one sync wait per
    instruction. Split any instruction carrying more onto single-wait
    NOPs inserted immediately before it on the same engine (in-order
    engines make this semantics-preserving)."""
    ctr = [0]

    def mk_nop(engine, wait):
        ctr[0] += 1
        n = mybir.InstNoOp(name=f"I-wsplit{ctr[0]}", ins=[], outs=[])
        n.engine = engine
        n.sync_info = bass_rust.SyncInfo(on_wait=[wait], on_update=[])
        return n

    for f in nc.m.functions:
        for bb in f.blocks:
            out = []
            changed = False
            for inst in bb.instructions:
                si = inst.sync_info
                waits = list(si.on_wait) if si is not None else []
                if len(waits) > 1:
                    for w in waits[:-1]:
                        out.append(mk_nop(inst.engine, w))
                    inst.sync_info = bass_rust.SyncInfo(
                        on_wait=[waits[-1]], on_update=list(si.on_update)
                    )
                    changed = True
                out.append(inst)
            if changed:
                bb.instructions = out


def build(split_waits=True):
    _install_tile_drain_fix()
    nc = bass.Bass("TRN2", target_bir_lowering=False, debug=False)

    x_ext = nc.declare_dram_parameter("x", [C, HW], F32, isOutput=False)
    w_ext = nc.declare_dram_parameter("Wqkv", [2 * M + C, C], F32, isOutput=False)
    b_ext = nc.declare_dram_parameter("bqkv", [2 * M + C, 1], F32, isOutput=False)
    g_ext = nc.declare_dram_parameter("gamma", [1, 1], F32, isOutput=False)
    out_ext = nc.declare_dram_parameter("out", [C, HW], F32, isOutput=True)

    ident_dram = nc.inline_tensor(np.eye(128, dtype=np.float32), "ident128")
    onesr_dram = nc.inline_tensor(np.ones((1, 128), dtype=np.float32), "onesrow")

    with tile.TileContext(nc) as tc:
        with (
            tc.tile_pool(name="const", bufs=1) as constp,
            tc.tile_pool(name="wld", bufs=1) as wldp,
            tc.tile_pool(name="wt", bufs=1) as wtp,
            tc.tile_pool(name="xin", bufs=1) as xp,
            tc.tile_pool(name="xr", bufs=1) as xrp,
            tc.tile_pool(name="qk", bufs=1) as qkp,
            tc.tile_pool(name="vt", bufs=1) as vtp,
            tc.tile_pool(name="e", bufs=6) as ep,
            tc.tile_pool(name="osb", bufs=2) as osbp,
            tc.tile_pool(name="misc", bufs=1) as miscp,
            # PSUM budget (16KB/partition = 8 banks):
            #   beta pairs [128,2,512]f32 x2 bufs = 8KB
            #   o_acc 2 x [128,512]f32          = 4KB
            #   epil (den [1,512] / rb [128,512]) x2 bufs = 4KB
            tc.tile_pool(name="ps_b", bufs=2, space="PSUM") as psb,
            tc.tile_pool(name="ps_acc", bufs=1, space="PSUM") as psacc,
            tc.tile_pool(name="ps_e", bufs=2, space="PSUM") as pse,
        ):
            # ---- constants ----
            ident = constp.tile([128, 128], F32)
            nc.sync.dma_start(ident[:], ident_dram.ap()[:, :])
            ones_row = constp.tile([1, 128], F32)
            nc.sync.dma_start(ones_row[:], onesr_dram.ap()[:, :])
            ones_row_bf = constp.tile([1, 128], BF16)
            nc.vector.tensor_copy(ones_row_bf[:], ones_row[:])
            ones_pair = constp.tile([128, 2, 32], F8)
            nc.vector.memset(ones_pair[:], 1.0)

            # ---- load W rows, biases, gamma ----
            w_rows = []
            for oc in range(3):
                wt_ = wldp.tile([128, C], F32, tag=f"wrows{oc}", name=f"wrows{oc}")
                nc.sync.dma_start(wt_[:], w_ext.ap()[128 * oc : 128 * (oc + 1), :])
                w_rows.append(wt_)
            bias_qq = miscp.tile([128, 1], F32, tag="bqq")
            nc.sync.dma_start(bias_qq[0:64, :], b_ext.ap()[0:64, :])
            nc.sync.dma_start(bias_qq[64:128, :], b_ext.ap()[0:64, :])
            bias_kk = miscp.tile([128, 1], F32, tag="bkk")
            nc.sync.dma_start(bias_kk[0:64, :], b_ext.ap()[64:128, :])
            nc.sync.dma_start(bias_kk[64:128, :], b_ext.ap()[64:128, :])
            bias_v = []
            for cc in range(2):
                bv = miscp.tile([128, 1], F32, tag=f"bv{cc}", name=f"bv{cc}")
                nc.sync.dma_start(
                    bv[:], b_ext.ap()[128 + 128 * cc : 128 + 128 * (cc + 1), :]
                )
                bias_v.append(bv)
            gamma_sb = miscp.tile([1, 1], F32, tag="gam")
            nc.sync.dma_start(gamma_sb[:], g_ext.ap()[:, :])

            gamma_bf = miscp.tile([1, 1], BF16, tag="gambf")
            nc.vector.tensor_copy(gamma_bf[:], gamma_sb[:])
            g_ps = psb.tile([128, 1], F32, tag="beta", name="g_ps")
            nc.tensor.matmul(g_ps[:], ones_row_bf[:], gamma_bf[:], start=True, stop=True)
            gamma_bc = miscp.tile([128, 1], F32, tag="gbc_sb")
            nc.vector.tensor_copy(gamma_bc[:], g_ps[:])
            gbv = []
            for cc in range(2):
                t = miscp.tile([128, 1], F32, tag=f"gbv{cc}", name=f"gbv{cc}")
                nc.vector.tensor_mul(t[:], bias_v[cc][:], gamma_bc[:])
                gbv.append(t)

            # ---- transpose W via PE, pack into fp8 DoubleRow layouts ----
            # wq_pack/wk_pack: [c%128, c-tile, m-dup] ; wv_pack: [c%128, c-tile, ch]
            wq_pack = wtp.tile([128, 2, 128], F8, tag="wq")
            wk_pack = wtp.tile([128, 2, 128], F8, tag="wk")
            wv_pack = wtp.tile([128, 2, 256], F8, tag="wv")
            for cc in range(2):
                ps = psb.tile([128, 128], F32, tag="beta", name="wtp_ps")
                nc.tensor.transpose(ps[:], w_rows[0][:, 128 * cc : 128 * (cc + 1)], ident[:])
                nc.vector.tensor_copy(wq_pack[:, cc, 0:64], ps[:, 0:64])
                nc.vector.tensor_copy(wq_pack[:, cc, 64:128], ps[:, 0:64])
                nc.vector.tensor_copy(wk_pack[:, cc, 0:64], ps[:, 64:128])
                nc.vector.tensor_copy(wk_pack[:, cc, 64:128], ps[:, 64:128])
            for oc in range(2):
                for cc in range(2):
                    ps = psb.tile([128, 128], F32, tag="beta", name="wtp_ps")
                    nc.tensor.transpose(
                        ps[:], w_rows[1 + oc][:, 128 * cc : 128 * (cc + 1)], ident[:]
                    )
                    nc.vector.tensor_copy(
                        wv_pack[:, cc, 128 * oc : 128 * (oc + 1)], ps[:]
                    )

            # ---- persistent SBUF tensors ----
            x_sb = [
                xp.tile([128, HW], F32, tag=f"x{cc}", name=f"xchunk{cc}")
                for cc in range(2)
            ]
            xr_pack = xrp.tile([128, 2, HW], F8, tag="xr")
            qq_sb = qkp.tile([128, HW], BF16, tag="qq")
            kk_sb = qkp.tile([128, HW], BF16, tag="kk")
            vtg = vtp.tile([128, N_IT, 256], F8, tag="vtg")

            PC = 2 * JC  # 1024-wide prologue steps

            def prologue(k):
                sl = slice(PC * k, PC * (k + 1))
                for cc in range(2):
                    nc.sync.dma_start(
                        x_sb[cc][:, sl], x_ext.ap()[128 * cc : 128 * (cc + 1), sl]
                    )
                    # relu + fp8 quantize on DVE (ACT is the exp bottleneck)
                    nc.vector.tensor_scalar_max(
                        xr_pack[:, cc, sl], x_sb[cc][:, sl], 0.0
                    )
                # k first (j-chunk 0 needs kk[:, 0:512] earliest)
                # one PSUM alloc per projection, two bank-aligned N=512
                # matmuls (a matmul output cannot span PSUM banks)
                kps = psb.tile([128, PC], F32, tag="beta", name="kps")
                for h in range(2):
                    hsl = slice(PC * k + JC * h, PC * k + JC * (h + 1))
                    nc.tensor.matmul(
                        kps[:, JC * h : JC * (h + 1)], wk_pack[:, :, :],
                        xr_pack[:, :, hsl], start=True, stop=True, perf_mode=DR,
                    )
                nc.vector.tensor_scalar_add(kk_sb[:, sl], kps[:], bias_kk[:])
                qps = psb.tile([128, PC], F32, tag="beta", name="qps")
                for h in range(2):
                    hsl = slice(PC * k + JC * h, PC * k + JC * (h + 1))
                    nc.tensor.matmul(
                        qps[:, JC * h : JC * (h + 1)], wq_pack[:, :, :],
                        xr_pack[:, :, hsl], start=True, stop=True, perf_mode=DR,
                    )
                nc.vector.tensor_scalar_add(qq_sb[:, sl], qps[:], bias_qq[:])
                # v for the 8 i-tiles of this step, batched 4 per PSUM alloc
                # (2 i-tiles share a 2KB bank: start on the first, fresh-byte
                # accumulate on the second)
                for q in range(2):
                    pq = psb.tile([128, 4, 256], F32, tag="beta", name="pq")
                    for t4 in range(4):
                        tt = 8 * k + 4 * q + t4
                        nc.tensor.matmul(
                            pq[:, t4, :],
                            xr_pack[:, :, 128 * tt : 128 * (tt + 1)],
                            wv_pack[:, :, :],
                            start=(t4 % 2 == 0), stop=(t4 % 2 == 1),
                            perf_mode=DR, skip_group_check=True,
                        )
                    nc.scalar.mul(
                        vtg[:, 8 * k + 4 * q : 8 * k + 4 * q + 4, :],
                        pq[:], gamma_bc[:],
                    )

            def attn_pair(jc, tp, o_acc, den):
                jsl = slice(JC * jc, JC * (jc + 1))
                it0, it1 = 2 * tp, 2 * tp + 1
                pst = psb.tile([128, 2, JC], F32, tag="beta", name="pst")
                nc.tensor.matmul(
                    pst[:, 0, :],
                    qq_sb[0:64, 128 * it0 : 128 * (it0 + 1)],
                    kk_sb[0:64, jsl],
                    start=True, stop=True,
                )
                nc.tensor.matmul(
                    pst[:, 1, :],
                    qq_sb[64:128, 128 * it1 : 128 * (it1 + 1)],
                    kk_sb[64:128, jsl],
                    start=True, stop=True,
                )
                e_t = ep.tile([128, 2, JC], F8, tag="e", name="et")
                nc.scalar.activation(e_t[:], pst[:], AF.Exp, scale=0.125)
                first = tp == 0
                last = tp == N_PAIR - 1
                for cc in range(2):
                    nc.tensor.matmul(
                        o_acc[cc][:],
                        vtg[:, it0 : it0 + 2, 128 * cc : 128 * (cc + 1)],
                        e_t[:],
                        start=first, stop=last, perf_mode=DR,
                    )
                nc.tensor.matmul(
                    den[:], ones_pair[:], e_t[:],
                    start=first, stop=last, perf_mode=DR,
                )

            def epilogue_a(jc, o_acc, den):
                # reciprocal chain first (it gates the rb matmul), then free
                # the PSUM accumulators
                den_sb = miscp.tile([1, JC], F32, tag="densb", bufs=2)
                nc.vector.tensor_copy(den_sb[:], den[0:1, :])
                rden = miscp.tile([1, JC], F32, tag="rden", bufs=2)
                nc.vector.reciprocal(rden[:], den_sb[:])
                rden_bf = miscp.tile([1, JC], BF16, tag="rdenbf", bufs=2)
                nc.vector.tensor_copy(rden_bf[:], rden[:])
                oc_sb = []
                for cc in range(2):
                    t = osbp.tile([128, JC], F32, tag=f"ocp{cc}", name=f"ocp{cc}")
                    nc.vector.tensor_copy(t[:], o_acc[cc][:])
                    oc_sb.append(t)
                return oc_sb, rden_bf

            def epilogue_b(jc, oc_sb, rden_bf):
                # emitted a couple of pairs into the next chunk so the
                # rb matmul never blocks the PE behind the reciprocal
                jsl = slice(JC * jc, JC * (jc + 1))
                rb_ps = pse.tile([128, JC], F32, tag="epil", name="rb_ps")
                nc.tensor.matmul(
                    rb_ps[:], ones_row_bf[:], rden_bf[:], start=True, stop=True
                )
                for cc in range(2):
                    o_n = osbp.tile([128, JC], F32, tag="on")
                    nc.vector.tensor_mul(o_n[:], oc_sb[cc][:], rb_ps[:])
                    res = osbp.tile([128, JC], F32, tag="res")
                    nc.vector.scalar_tensor_tensor(
                        res[:],
                        in0=o_n[:],
                        scalar=gbv[cc][:],
                        in1=x_sb[cc][:, jsl],
                        op0=mybir.AluOpType.add,
                        op1=mybir.AluOpType.add,
                    )
                    nc.sync.dma_start(
                        out_ext.ap()[128 * cc : 128 * (cc + 1), jsl], res[:]
                    )

            # ---- emission: prologue runs 2 chunks ahead of the j-chunk-0
            # pairs that consume it; epilogues split across the boundary ----
            def new_acc():
                o_acc = [
                    psacc.tile([128, JC], F32, tag=f"oacc{cc}", name=f"oacc{cc}")
                    for cc in range(2)
                ]
                den = pse.tile([32, JC], F32, tag="epil", name="den")
                return o_acc, den

            prologue(0)
            prologue(1)
            o_acc0, den0 = new_acc()
            for tp in range(4):
                attn_pair(0, tp, o_acc0, den0)
            prologue(2)
            for tp in range(4, 8):
                attn_pair(0, tp, o_acc0, den0)
            prologue(3)
            for tp in range(8, N_PAIR):
                attn_pair(0, tp, o_acc0, den0)
            carry = epilogue_a(0, o_acc0, den0)

            for jc in range(1, N_JC):
                o_acc, den = new_acc()
                for tp in range(5):
                    attn_pair(jc, tp, o_acc, den)
                epilogue_b(jc - 1, *carry)
                for tp in range(5, N_PAIR):
                    attn_pair(jc, tp, o_acc, den)
                carry = epilogue_a(jc, o_acc, den)
            epilogue_b(N_JC - 1, *carry)

    if split_waits:
        _split_multi_waits(nc)
    return nc


_NC_CACHE = None


def kernel(x, Wqkv, bqkv, gamma):
    global _NC_CACHE
    if _NC_CACHE is None:
        _NC_CACHE = build()
    nc = _NC_CACHE
    B = x.shape[0]
    assert B == N_CORES
    in_maps = []
    for i in range(B):
        in_maps.append(
            {
                "x": np.ascontiguousarray(x[i].reshape(C, HW), dtype=np.float32),
                "Wqkv": np.ascontiguousarray(Wqkv, dtype=np.float32),
                "bqkv": np.ascontiguousarray(np.asarray(bqkv).reshape(2 * M + C, 1), dtype=np.float32),
                "gamma": np.ascontiguousarray(np.asarray(gamma).reshape(1, 1), dtype=np.float32),
            }
        )
    res = run_bass_kernel_spmd(nc, in_maps, core_ids=list(range(N_CORES)))
    out = np.stack(
        [res.results[i]["out"].reshape(C, 64, 64) for i in range(N_CORES)]
    ).astype(np.float32)
    return out


# revision 3
# speedup vs baseline: 1.0186x; 1.0186x over previous
"""Trainium2 Bass kernel for nn_AttentionLayer (final, ~224 us vs 425 us baseline).

Data-parallel over batch: each of the 8 NeuronCores runs one image
[256, 64*64] end-to-end (QKV 1x1-conv, q^T k scores, softmax over the
query axis, v @ P, residual) with the small weights replicated.

Key design points (arrived at via perfetto-trace iteration):
  - fp8(e4m3) DoubleRow matmuls for the QKV projection, the PV
    accumulation and the ones-row den reduction: 2 contraction k-tiles
    (256 rows) per 512-column pass, halving PE streaming work vs bf16.
    Scores stay bf16 as two concurrent K=64 row-group matmuls (a
    64-deep contraction cannot use DoubleRow).
  - score PSUM tiles are [128, 2, 512] pairs: ONE exp per pair on the
    scalar engine (the bottleneck: 128 exps x ~1.11 us), writing fp8
    in exactly the [K, 2, N] layout DoubleRow consumes downstream.
  - softmax denominator via a DoubleRow ones-matmul into a [32, 512]
    PSUM row; 1/den on DVE off the critical path, broadcast across
    partitions with a rank-1 bf16 matmul.
  - PSUM budget exactly 8 banks: score pairs 2x2, o_acc 2, den/rb 2.
  - prologue (DMA, relu+fp8 quantize on DVE, QKV, vtg quantize on ACT)
    runs in 1024-column steps software-pipelined 2 steps ahead of the
    j-chunk-0 attention pairs so the exp stream starts ~10 us in;
    v-projection outputs are batched 4 i-tiles per PSUM allocation to
    limit tile-pool rotation stalls.
  - per-chunk epilogue split: PSUM-freeing copies + reciprocal at the
    chunk boundary, normalize/residual/DMA-out emitted 5 pairs into
    the next chunk so the broadcast matmul never stalls the PE queue.

Known hardware context: the PE runs at an effective ~1.4 GHz here
(power throttling caps utilization ~25% of the time); steady-state is
PE/ACT co-limited at ~1.29/1.11 us per i-tile pair.
"""

import numpy as np

import bass_rust
import concourse.bass as bass
import concourse.tile as tile
from concourse import mybir
from concourse.bass_utils import run_bass_kernel_spmd

N_CORES = 8
C = 256
M = 64
HW = 4096
JC = 512
N_JC = HW // JC          # 8 j-chunks
N_IT = HW // 128         # 32 i-tiles
N_PAIR = N_IT // 2       # 16 i-tile pairs per j-chunk

F32 = mybir.dt.float32
BF16 = mybir.dt.bfloat16
F8 = mybir.dt.float8e4
DR = mybir.MatmulPerfMode.DoubleRow
AF = mybir.ActivationFunctionType


def _install_tile_drain_fix():
    def _drain_and_barrier(self, tick_clock, wait_clock):
        from concourse.tile import ScopedClock

        nc = self.nc
        probe = nc.sync.nop()
        wait_clock.add_sem_waits(
            probe.ins, ScopedClock({None: tick_clock.global_clock})
        )
        si = probe.ins.sync_info
        waits = list(si.on_wait) if si is not None else []
        probe.ins.sync_info = bass_rust.SyncInfo(on_wait=waits[:1], on_update=[])
        for w in waits[1:]:
            n = nc.sync.nop()
            n.ins.sync_info = bass_rust.SyncInfo(on_wait=[w], on_update=[])
        nc.sync.drain()
        nc.all_engine_barrier()
        assert self.sems is not None
        popped = nc._tile_sem_poison_stack.pop()
        assert popped is self._sem_poison
        nc.clear_and_free_semaphores(list(self.sems.allocated().values()))
        nc.all_engine_barrier()

    tile.TileContext._drain_and_barrier = _drain_and_barrier


def _split_multi_waits(nc):
    """walrus in this toolchain encodes at most one sync wait per
    instruction. Split any instruction carrying more onto single-wait
    NOPs inserted immediately before it on the same engine (in-order
    engines make this semantics-preserving)."""
    ctr = [0]

    def mk_nop(engine, wait):
        ctr[0] += 1
        n = mybir.InstNoOp(name=f"I-wsplit{ctr[0]}", ins=[], outs=[])
        n.engine = engine
        n.sync_info = bass_rust.SyncInfo(on_wait=[wait], on_update=[])
        return n

    for f in nc.m.functions:
        for bb in f.blocks:
            out = []
            changed = False
            for inst in bb.instructions:
                si = inst.sync_info
                waits = list(si.on_wait) if si is not None else []
                if len(waits) > 1:
                    for w in waits[:-1]:
                        out.append(mk_nop(inst.engine, w))
                    inst.sync_info = bass_rust.SyncInfo(
                        on_wait=[waits[-1]], on_update=list(si.on_update)
                    )
                    changed = True
                out.append(inst)
            if changed:
                bb.instructions = out


def build(split_waits=True):
    _install_tile_drain_fix()
    nc = bass.Bass("TRN2", target_bir_lowering=False, debug=False)

    x_ext = nc.declare_dram_parameter("x", [C, HW], F32, isOutput=False)
    w_ext = nc.declare_dram_parameter("Wqkv", [2 * M + C, C], F32, isOutput=False)
    b_ext = nc.declare_dram_parameter("bqkv", [2 * M + C, 1], F32, isOutput=False)
    g_ext = nc.declare_dram_parameter("gamma", [1, 1], F32, isOutput=False)
    out_ext = nc.declare_dram_parameter("out", [C, HW], F32, isOutput=True)

    ident_dram = nc.inline_tensor(np.eye(128, dtype=np.float32), "ident128")
    onesr_dram = nc.inline_tensor(np.ones((1, 128), dtype=np.float32), "onesrow")

    with tile.TileContext(nc) as tc:
        with (
            tc.tile_pool(name="const", bufs=1) as constp,
            tc.tile_pool(name="wld", bufs=1) as wldp,
            tc.tile_pool(name="wt", bufs=1) as wtp,
            tc.tile_pool(name="xin", bufs=1) as xp,
            tc.tile_pool(name="xr", bufs=1) as xrp,
            tc.tile_pool(name="qk", bufs=1) as qkp,
            tc.tile_pool(name="vt", bufs=1) as vtp,
            tc.tile_pool(name="e", bufs=6) as ep,
            tc.tile_pool(name="osb", bufs=2) as osbp,
            tc.tile_pool(name="misc", bufs=1) as miscp,
            # PSUM budget (16KB/partition = 8 banks):
            #   beta pairs [128,2,512]f32 x2 bufs = 8KB
            #   o_acc 2 x [128,512]f32          = 4KB
            #   epil (den [1,512] / rb [128,512]) x2 bufs = 4KB
            tc.tile_pool(name="ps_b", bufs=2, space="PSUM") as psb,
            tc.tile_pool(name="ps_acc", bufs=1, space="PSUM") as psacc,
            tc.tile_pool(name="ps_e", bufs=2, space="PSUM") as pse,
        ):
            # ---- constants ----
            ident = constp.tile([128, 128], F32)
            nc.sync.dma_start(ident[:], ident_dram.ap()[:, :])
            ones_row = constp.tile([1, 128], F32)
            nc.sync.dma_start(ones_row[:], onesr_dram.ap()[:, :])
            ones_row_bf = constp.tile([1, 128], BF16)
            nc.vector.tensor_copy(ones_row_bf[:], ones_row[:])
            ones_pair = constp.tile([128, 2, 32], F8)
            nc.vector.memset(ones_pair[:], 1.0)

            # ---- load W rows, biases, gamma ----
            w_rows = []
            for oc in range(3):
                wt_ = wldp.tile([128, C], F32, tag=f"wrows{oc}", name=f"wrows{oc}")
                nc.sync.dma_start(wt_[:], w_ext.ap()[128 * oc : 128 * (oc + 1), :])
                w_rows.append(wt_)
            bias_qq = miscp.tile([128, 1], F32, tag="bqq")
            nc.sync.dma_start(bias_qq[0:64, :], b_ext.ap()[0:64, :])
            nc.sync.dma_start(bias_qq[64:128, :], b_ext.ap()[0:64, :])
            bias_kk = miscp.tile([128, 1], F32, tag="bkk")
            nc.sync.dma_start(bias_kk[0:64, :], b_ext.ap()[64:128, :])
            nc.sync.dma_start(bias_kk[64:128, :], b_ext.ap()[64:128, :])
            bias_v = []
            for cc in range(2):
                bv = miscp.tile([128, 1], F32, tag=f"bv{cc}", name=f"bv{cc}")
                nc.sync.dma_start(
                    bv[:], b_ext.ap()[128 + 128 * cc : 128 + 128 * (cc + 1), :]
                )
                bias_v.append(bv)
            gamma_sb = miscp.tile([1, 1], F32, tag="gam")
            nc.sync.dma_start(gamma_sb[:], g_ext.ap()[:, :])

            gamma_bf = miscp.tile([1, 1], BF16, tag="gambf")
            nc.vector.tensor_copy(gamma_bf[:], gamma_sb[:])
            g_ps = psb.tile([128, 1], F32, tag="beta", name="g_ps")
            nc.tensor.matmul(g_ps[:], ones_row_bf[:], gamma_bf[:], start=True, stop=True)
            gamma_bc = miscp.tile([128, 1], F32, tag="gbc_sb")
            nc.vector.tensor_copy(gamma_bc[:], g_ps[:])
            gbv = []
            for cc in range(2):
                t = miscp.tile([128, 1], F32, tag=f"gbv{cc}", name=f"gbv{cc}")
                nc.vector.tensor_mul(t[:], bias_v[cc][:], gamma_bc[:])
                gbv.append(t)

            # ---- transpose W via PE, pack into fp8 DoubleRow layouts ----
            # wq_pack/wk_pack: [c%128, c-tile, m-dup] ; wv_pack: [c%128, c-tile, ch]
            wq_pack = wtp.tile([128, 2, 128], F8, tag="wq")
            wk_pack = wtp.tile([128, 2, 128], F8, tag="wk")
            wv_pack = wtp.tile([128, 2, 256], F8, tag="wv")
            for cc in range(2):
                ps = psb.tile([128, 128], F32, tag="beta", name="wtp_ps")
                nc.tensor.transpose(ps[:], w_rows[0][:, 128 * cc : 128 * (cc + 1)], ident[:])
                nc.vector.tensor_copy(wq_pack[:, cc, 0:64], ps[:, 0:64])
                nc.vector.tensor_copy(wq_pack[:, cc, 64:128], ps[:, 0:64])
                nc.vector.tensor_copy(wk_pack[:, cc, 0:64], ps[:, 64:128])
                nc.vector.tensor_copy(wk_pack[:, cc, 64:128], ps[:, 64:128])
            for oc in range(2):
                for cc in range(2):
                    ps = psb.tile([128, 128], F32, tag="beta", name="wtp_ps")
                    nc.tensor.transpose(
                        ps[:], w_rows[1 + oc][:, 128 * cc : 128 * (cc + 1)], ident[:]
                    )
                    nc.vector.tensor_copy(
                        wv_pack[:, cc, 128 * oc : 128 * (oc + 1)], ps[:]
                    )

            # ---- persistent SBUF tensors ----
            x_sb = [
                xp.tile([128, HW], F32, tag=f"x{cc}", name=f"xchunk{cc}")
                for cc in range(2)
            ]
            xr_pack = xrp.tile([128, 2, HW], F8, tag="xr")
            qq_sb = qkp.tile([128, HW], BF16, tag="qq")
            kk_sb = qkp.tile([128, HW], BF16, tag="kk")
            vtg = vtp.tile([128, N_IT, 256], F8, tag="vtg")

            PC = 2 * JC  # 1024-wide prologue steps

            def prologue(k):
                sl = slice(PC * k, PC * (k + 1))
                for cc in range(2):
                    nc.sync.dma_start(
                        x_sb[cc][:, sl], x_ext.ap()[128 * cc : 128 * (cc + 1), sl]
                    )
                    # relu + fp8 quantize on DVE (ACT is the exp bottleneck)
                    nc.vector.tensor_scalar_max(
                        xr_pack[:, cc, sl], x_sb[cc][:, sl], 0.0
                    )
                # k first (j-chunk 0 needs kk[:, 0:512] earliest)
                # one PSUM alloc per projection, two bank-aligned N=512
                # matmuls (a matmul output cannot span PSUM banks)
                kps = psb.tile([128, PC], F32, tag="beta", name="kps")
                for h in range(2):
                    hsl = slice(PC * k + JC * h, PC * k + JC * (h + 1))
                    nc.tensor.matmul(
                        kps[:, JC * h : JC * (h + 1)], wk_pack[:, :, :],
                        xr_pack[:, :, hsl], start=True, stop=True, perf_mode=DR,
                    )
                nc.vector.tensor_scalar_add(kk_sb[:, sl], kps[:], bias_kk[:])
                qps = psb.tile([128, PC], F32, tag="beta", name="qps")
                for h in range(2):
                    hsl = slice(PC * k + JC * h, PC * k + JC * (h + 1))
                    nc.tensor.matmul(
                        qps[:, JC * h : JC * (h + 1)], wq_pack[:, :, :],
                        xr_pack[:, :, hsl], start=True, stop=True, perf_mode=DR,
                    )
                nc.vector.tensor_scalar_add(qq_sb[:, sl], qps[:], bias_qq[:])
                # v for the 8 i-tiles of this step, batched 4 per PSUM alloc
                # (2 i-tiles share a 2KB bank: start on the first, fresh-byte
                # accumulate on the second)
                for q in range(2):
                    pq = psb.tile([128, 4, 256], F32, tag="beta", name="pq")
                    for t4 in range(4):
                        tt = 8 * k + 4 * q + t4
                        nc.tensor.matmul(
                            pq[:, t4, :],
                            xr_pack[:, :, 128 * tt : 128 * (tt + 1)],
                            wv_pack[:, :, :],
                            start=(t4 % 2 == 0), stop=(t4 % 2 == 1),
                            perf_mode=DR, skip_group_check=True,
                        )
                    nc.scalar.mul(
                        vtg[:, 8 * k + 4 * q : 8 * k + 4 * q + 4, :],
                        pq[:], gamma_bc[:],
                    )

            def attn_pair(jc, tp, o_acc, den):
                jsl = slice(JC * jc, JC * (jc + 1))
                it0, it1 = 2 * tp, 2 * tp + 1
                pst = psb.tile([128, 2, JC], F32, tag="beta", name="pst")
                nc.tensor.matmul(
                    pst[:, 0, :],
                    qq_sb[0:64, 128 * it0 : 128 * (it0 + 1)],
                    kk_sb[0:64, jsl],
                    start=True, stop=True,
                )
                nc.tensor.matmul(
                    pst[:, 1, :],
                    qq_sb[64:128, 128 * it1 : 128 * (it1 + 1)],
                    kk_sb[64:128, jsl],
                    start=True, stop=True,
                )
                e_t = ep.tile([128, 2, JC], F8, tag="e", name="et")
                nc.scalar.activation(e_t[:], pst[:], AF.Exp, scale=0.125)
                first = tp == 0
                last = tp == N_PAIR - 1
                for cc in range(2):
                    nc.tensor.matmul(
                        o_acc[cc][:],
                        vtg[:, it0 : it0 + 2, 128 * cc : 128 * (cc + 1)],
                        e_t[:],
                        start=first, stop=last, perf_mode=DR,
                    )
                nc.tensor.matmul(
                    den[:], ones_pair[:], e_t[:],
                    start=first, stop=last, perf_mode=DR,
                )

            def epilogue_a(jc, o_acc, den):
                # reciprocal chain first (it gates the rb matmul), then free
                # the PSUM accumulators
                den_sb = miscp.tile([1, JC], F32, tag="densb", bufs=2)
                nc.vector.tensor_copy(den_sb[:], den[0:1, :])
                rden = miscp.tile([1, JC], F32, tag="rden", bufs=2)
                nc.vector.reciprocal(rden[:], den_sb[:])
                rden_bf = miscp.tile([1, JC], BF16, tag="rdenbf", bufs=2)
                nc.vector.tensor_copy(rden_bf[:], rden[:])
                oc_sb = []
                for cc in range(2):
                    t = osbp.tile([128, JC], F32, tag=f"ocp{cc}", name=f"ocp{cc}")
                    nc.vector.tensor_copy(t[:], o_acc[cc][:])
                    oc_sb.append(t)
                return oc_sb, rden_bf

            def epilogue_b(jc, oc_sb, rden_bf):
                # emitted a couple of pairs into the next chunk so the
                # rb matmul never blocks the PE behind the reciprocal
                jsl = slice(JC * jc, JC * (jc + 1))
                rb_ps = pse.tile([128, JC], F32, tag="epil", name="rb_ps")
                nc.tensor.matmul(
                    rb_ps[:], ones_row_bf[:], rden_bf[:], start=True, stop=True
                )
                for cc in range(2):
                    o_n = osbp.tile([128, JC], F32, tag="on")
                    nc.vector.tensor_mul(o_n[:], oc_sb[cc][:], rb_ps[:])
                    res = osbp.tile([128, JC], F32, tag="res")
                    nc.vector.scalar_tensor_tensor(
                        res[:],
                        in0=o_n[:],
                        scalar=gbv[cc][:],
                        in1=x_sb[cc][:, jsl],
                        op0=mybir.AluOpType.add,
                        op1=mybir.AluOpType.add,
                    )
                    nc.sync.dma_start(
                        out_ext.ap()[128 * cc : 128 * (cc + 1), jsl], res[:]
                    )

            # ---- emission: prologue runs 2 chunks ahead of the j-chunk-0
            # pairs that consume it; epilogues split across the boundary ----
            def new_acc():
                o_acc = [
                    psacc.tile([128, JC], F32, tag=f"oacc{cc}", name=f"oacc{cc}")
                    for cc in range(2)
                ]
                den = pse.tile([32, JC], F32, tag="epil", name="den")
                return o_acc, den

            prologue(0)
            prologue(1)
            o_acc0, den0 = new_acc()
            for tp in range(4):
                attn_pair(0, tp, o_acc0, den0)
            prologue(2)
            for tp in range(4, 8):
                attn_pair(0, tp, o_acc0, den0)
            prologue(3)
            for tp in range(8, N_PAIR):
                attn_pair(0, tp, o_acc0, den0)
            carry = epilogue_a(0, o_acc0, den0)

            for jc in range(1, N_JC):
                o_acc, den = new_acc()
                for tp in range(5):
                    attn_pair(jc, tp, o_acc, den)
                epilogue_b(jc - 1, *carry)
                for tp in range(5, N_PAIR):
                    attn_pair(jc, tp, o_acc, den)
                carry = epilogue_a(jc, o_acc, den)
            epilogue_b(N_JC - 1, *carry)

    if split_waits:
        _split_multi_waits(nc)
    return nc


_NC_CACHE = None


def kernel(x, Wqkv, bqkv, gamma):
    global _NC_CACHE
    if _NC_CACHE is None:
        _NC_CACHE = build()
    nc = _NC_CACHE
    B = x.shape[0]
    assert B == N_CORES
    in_maps = []
    for i in range(B):
        in_maps.append(
            {
                "x": np.ascontiguousarray(x[i].reshape(C, HW), dtype=np.float32),
                "Wqkv": np.ascontiguousarray(Wqkv, dtype=np.float32),
                "bqkv": np.ascontiguousarray(np.asarray(bqkv).reshape(2 * M + C, 1), dtype=np.float32),
                "gamma": np.ascontiguousarray(np.asarray(gamma).reshape(1, 1), dtype=np.float32),
            }
        )
    res = run_bass_kernel_spmd(nc, in_maps, core_ids=list(range(N_CORES)))
    out = np.stack(
        [res.results[i]["out"].reshape(C, 64, 64) for i in range(N_CORES)]
    ).astype(np.float32)
    return out


# revision 4
# speedup vs baseline: 1.0321x; 1.0133x over previous
"""Trainium2 Bass kernel for nn_AttentionLayer. v13

Changes from v4:
  - fp8 (e4m3) DoubleRow matmuls for QKV projection, PV, and den:
    0.5 cycles/row on the PE (2x bf16).  Scores stay bf16 (K=64
    row-packed pairs; DoubleRow cannot help a 64-deep contraction).
  - score PSUM tiles are [128, 2, 512] pairs; ONE exp per pair
    (halves ACT per-instruction overhead) writing fp8 directly in the
    [K, 2, N] layout DoubleRow consumes.
  - den computed per-pair via a DoubleRow ones-matmul (one [1,512]
    row, no col-group halves to add).
  - reciprocal_approx_fast (custom DVE op, ~5x faster than
    reciprocal) for the softmax denominator.
  - prologue chunks interleaved with j-chunk-0 attention pairs so the
    ACT exp stream (the bottleneck engine) starts ~3us in instead of
    after the full prologue.  relu runs on ACT, qkv bias-copies and
    vtg quantize on DVE, to balance the two engines in that phase.
"""

import numpy as np

import bass_rust
import concourse.bass as bass
import concourse.tile as tile
from concourse import mybir
from concourse.bass_utils import run_bass_kernel_spmd

N_CORES = 8
C = 256
M = 64
HW = 4096
JC = 512
N_JC = HW // JC          # 8 j-chunks
N_IT = HW // 128         # 32 i-tiles
N_PAIR = N_IT // 2       # 16 i-tile pairs per j-chunk

F32 = mybir.dt.float32
BF16 = mybir.dt.bfloat16
F8 = mybir.dt.float8e4
DR = mybir.MatmulPerfMode.DoubleRow
AF = mybir.ActivationFunctionType


def _install_tile_drain_fix():
    def _drain_and_barrier(self, tick_clock, wait_clock):
        from concourse.tile import ScopedClock

        nc = self.nc
        probe = nc.sync.nop()
        wait_clock.add_sem_waits(
            probe.ins, ScopedClock({None: tick_clock.global_clock})
        )
        si = probe.ins.sync_info
        waits = list(si.on_wait) if si is not None else []
        probe.ins.sync_info = bass_rust.SyncInfo(on_wait=waits[:1], on_update=[])
        for w in waits[1:]:
            n = nc.sync.nop()
            n.ins.sync_info = bass_rust.SyncInfo(on_wait=[w], on_update=[])
        nc.sync.drain()
        nc.all_engine_barrier()
        assert self.sems is not None
        popped = nc._tile_sem_poison_stack.pop()
        assert popped is self._sem_poison
        nc.clear_and_free_semaphores(list(self.sems.allocated().values()))
        nc.all_engine_barrier()

    tile.TileContext._drain_and_barrier = _drain_and_barrier


def _split_multi_waits(nc):
    """walrus in this toolchain encodes at most one sync wait per
    instruction. Split any instruction carrying more onto single-wait
    NOPs inserted immediately before it on the same engine (in-order
    engines make this semantics-preserving)."""
    ctr = [0]

    def mk_nop(engine, wait):
        ctr[0] += 1
        n = mybir.InstNoOp(name=f"I-wsplit{ctr[0]}", ins=[], outs=[])
        n.engine = engine
        n.sync_info = bass_rust.SyncInfo(on_wait=[wait], on_update=[])
        return n

    for f in nc.m.functions:
        for bb in f.blocks:
            out = []
            changed = False
            for inst in bb.instructions:
                si = inst.sync_info
                waits = list(si.on_wait) if si is not None else []
                if len(waits) > 1:
                    for w in waits[:-1]:
                        out.append(mk_nop(inst.engine, w))
                    inst.sync_info = bass_rust.SyncInfo(
                        on_wait=[waits[-1]], on_update=list(si.on_update)
                    )
                    changed = True
                out.append(inst)
            if changed:
                bb.instructions = out


def build(split_waits=True):
    _install_tile_drain_fix()
    nc = bass.Bass("TRN2", target_bir_lowering=False, debug=False)

    x_ext = nc.declare_dram_parameter("x", [C, HW], F32, isOutput=False)
    w_ext = nc.declare_dram_parameter("Wqkv", [2 * M + C, C], F32, isOutput=False)
    b_ext = nc.declare_dram_parameter("bqkv", [2 * M + C, 1], F32, isOutput=False)
    g_ext = nc.declare_dram_parameter("gamma", [1, 1], F32, isOutput=False)
    out_ext = nc.declare_dram_parameter("out", [C, HW], F32, isOutput=True)

    ident_dram = nc.inline_tensor(np.eye(128, dtype=np.float32), "ident128")
    onesr_dram = nc.inline_tensor(np.ones((1, 128), dtype=np.float32), "onesrow")

    with tile.TileContext(nc) as tc:
        with (
            tc.tile_pool(name="const", bufs=1) as constp,
            tc.tile_pool(name="wld", bufs=1) as wldp,
            tc.tile_pool(name="wt", bufs=1) as wtp,
            tc.tile_pool(name="xin", bufs=1) as xp,
            tc.tile_pool(name="xr", bufs=1) as xrp,
            tc.tile_pool(name="qk", bufs=1) as qkp,
            tc.tile_pool(name="vt", bufs=1) as vtp,
            tc.tile_pool(name="e", bufs=6) as ep,
            tc.tile_pool(name="osb", bufs=2) as osbp,
            tc.tile_pool(name="misc", bufs=1) as miscp,
            # PSUM budget (16KB/partition = 8 banks):
            #   beta pairs [128,2,512]f32 x2 bufs = 8KB
            #   o_acc 2 x [128,512]f32          = 4KB
            #   epil (den [1,512] / rb [128,512]) x2 bufs = 4KB
            tc.tile_pool(name="ps_b", bufs=2, space="PSUM") as psb,
            tc.tile_pool(name="ps_acc", bufs=1, space="PSUM") as psacc,
            tc.tile_pool(name="ps_e", bufs=2, space="PSUM") as pse,
            tc.tile_pool(name="drscratch", bufs=2, space="DRAM") as drp,
        ):
            # ---- constants ----
            ident = constp.tile([128, 128], F32)
            nc.sync.dma_start(ident[:], ident_dram.ap()[:, :])
            ones_row = constp.tile([1, 128], F32)
            nc.sync.dma_start(ones_row[:], onesr_dram.ap()[:, :])
            ones_row_bf = constp.tile([1, 128], BF16)
            nc.vector.tensor_copy(ones_row_bf[:], ones_row[:])
            ones_pair = constp.tile([128, 2, 32], F8)
            nc.vector.memset(ones_pair[:], 1.0)

            # ---- load W rows, biases, gamma ----
            w_rows = []
            for oc in range(3):
                wt_ = wldp.tile([128, C], F32, tag=f"wrows{oc}", name=f"wrows{oc}")
                nc.sync.dma_start(wt_[:], w_ext.ap()[128 * oc : 128 * (oc + 1), :])
                w_rows.append(wt_)
            bias_qq = miscp.tile([128, 1], F32, tag="bqq")
            nc.sync.dma_start(bias_qq[0:64, :], b_ext.ap()[0:64, :])
            nc.sync.dma_start(bias_qq[64:128, :], b_ext.ap()[0:64, :])
            bias_kk = miscp.tile([128, 1], F32, tag="bkk")
            nc.sync.dma_start(bias_kk[0:64, :], b_ext.ap()[64:128, :])
            nc.sync.dma_start(bias_kk[64:128, :], b_ext.ap()[64:128, :])
            bias_v = []
            for cc in range(2):
                bv = miscp.tile([128, 1], F32, tag=f"bv{cc}", name=f"bv{cc}")
                nc.sync.dma_start(
                    bv[:], b_ext.ap()[128 + 128 * cc : 128 + 128 * (cc + 1), :]
                )
                bias_v.append(bv)
            gamma_sb = miscp.tile([1, 1], F32, tag="gam")
            nc.sync.dma_start(gamma_sb[:], g_ext.ap()[:, :])

            gamma_bf = miscp.tile([1, 1], BF16, tag="gambf")
            nc.vector.tensor_copy(gamma_bf[:], gamma_sb[:])
            g_ps = psb.tile([128, 1], F32, tag="beta", name="g_ps")
            nc.tensor.matmul(g_ps[:], ones_row_bf[:], gamma_bf[:], start=True, stop=True)
            gamma_bc = miscp.tile([128, 1], F32, tag="gbc_sb")
            nc.vector.tensor_copy(gamma_bc[:], g_ps[:])
            gbv = []
            for cc in range(2):
                t = miscp.tile([128, 1], F32, tag=f"gbv{cc}", name=f"gbv{cc}")
                nc.vector.tensor_mul(t[:], bias_v[cc][:], gamma_bc[:])
                gbv.append(t)

            # ---- transpose W via PE, pack into fp8 DoubleRow layouts ----
            # wq_pack/wk_pack: [c%128, c-tile, m-dup] ; wv_pack: [c%128, c-tile, ch]
            wq_pack = wtp.tile([128, 2, 128], F8, tag="wq")
            wk_pack = wtp.tile([128, 2, 128], F8, tag="wk")
            wv_pack = wtp.tile([128, 2, 256], F8, tag="wv")
            for cc in range(2):
                ps = psb.tile([128, 128], F32, tag="beta", name="wtp_ps")
                nc.tensor.transpose(ps[:], w_rows[0][:, 128 * cc : 128 * (cc + 1)], ident[:])
                nc.vector.tensor_copy(wq_pack[:, cc, 0:64], ps[:, 0:64])
                nc.vector.tensor_copy(wq_pack[:, cc, 64:128], ps[:, 0:64])
                nc.vector.tensor_copy(wk_pack[:, cc, 0:64], ps[:, 64:128])
                nc.vector.tensor_copy(wk_pack[:, cc, 64:128], ps[:, 64:128])
            for oc in range(2):
                for cc in range(2):
                    ps = psb.tile([128, 128], F32, tag="beta", name="wtp_ps")
                    nc.tensor.transpose(
                        ps[:], w_rows[1 + oc][:, 128 * cc : 128 * (cc + 1)], ident[:]
                    )
                    nc.vector.tensor_copy(
                        wv_pack[:, cc, 128 * oc : 128 * (oc + 1)], ps[:]
                    )

            # ---- persistent SBUF tensors ----
            x_sb = [
                xp.tile([128, HW], F32, tag=f"x{cc}", name=f"xchunk{cc}")
                for cc in range(2)
            ]
            xr_pack = xrp.tile([128, 2, HW], F8, tag="xr")
            qq_sb = qkp.tile([128, HW], BF16, tag="qq")
            kk_sb = qkp.tile([128, HW], BF16, tag="kk")
            vtg = vtp.tile([128, N_IT, 256], F8, tag="vtg")

            PC = 2 * JC  # 1024-wide prologue steps

            def prologue(k):
                sl = slice(PC * k, PC * (k + 1))
                for cc in range(2):
                    nc.sync.dma_start(
                        x_sb[cc][:, sl], x_ext.ap()[128 * cc : 128 * (cc + 1), sl]
                    )
                    # relu + fp8 quantize on DVE (ACT is the exp bottleneck)
                    nc.vector.tensor_scalar_max(
                        xr_pack[:, cc, sl], x_sb[cc][:, sl], 0.0
                    )
                # k first (j-chunk 0 needs kk[:, 0:512] earliest)
                # one PSUM alloc per projection, two bank-aligned N=512
                # matmuls (a matmul output cannot span PSUM banks)
                kps = psb.tile([128, PC], F32, tag="beta", name="kps")
                for h in range(2):
                    hsl = slice(PC * k + JC * h, PC * k + JC * (h + 1))
                    nc.tensor.matmul(
                        kps[:, JC * h : JC * (h + 1)], wk_pack[:, :, :],
                        xr_pack[:, :, hsl], start=True, stop=True, perf_mode=DR,
                    )
                nc.vector.tensor_scalar_add(kk_sb[:, sl], kps[:], bias_kk[:])
                qps = psb.tile([128, PC], F32, tag="beta", name="qps")
                for h in range(2):
                    hsl = slice(PC * k + JC * h, PC * k + JC * (h + 1))
                    nc.tensor.matmul(
                        qps[:, JC * h : JC * (h + 1)], wq_pack[:, :, :],
                        xr_pack[:, :, hsl], start=True, stop=True, perf_mode=DR,
                    )
                nc.vector.tensor_scalar_add(qq_sb[:, sl], qps[:], bias_qq[:])
                # v for the 8 i-tiles of this step, batched 4 per PSUM alloc
                # (2 i-tiles share a 2KB bank: start on the first, fresh-byte
                # accumulate on the second)
                for q in range(2):
                    pq = psb.tile([128, 4, 256], F32, tag="beta", name="pq")
                    for t4 in range(4):
                        tt = 8 * k + 4 * q + t4
                        nc.tensor.matmul(
                            pq[:, t4, :],
                            xr_pack[:, :, 128 * tt : 128 * (tt + 1)],
                            wv_pack[:, :, :],
                            start=(t4 % 2 == 0), stop=(t4 % 2 == 1),
                            perf_mode=DR, skip_group_check=True,
                        )
                    nc.scalar.mul(
                        vtg[:, 8 * k + 4 * q : 8 * k + 4 * q + 4, :],
                        pq[:], gamma_bc[:],
                    )

            def attn_pair(jc, tp, o_acc, den):
                jsl = slice(JC * jc, JC * (jc + 1))
                it0, it1 = 2 * tp, 2 * tp + 1
                pst = psb.tile([128, 2, JC], F32, tag="beta", name="pst")
                nc.tensor.matmul(
                    pst[:, 0, :],
                    qq_sb[0:64, 128 * it0 : 128 * (it0 + 1)],
                    kk_sb[0:64, jsl],
                    start=True, stop=True,
                )
                nc.tensor.matmul(
                    pst[:, 1, :],
                    qq_sb[64:128, 128 * it1 : 128 * (it1 + 1)],
                    kk_sb[64:128, jsl],
                    start=True, stop=True,
                )
                e_t = ep.tile([128, 2, JC], F8, tag="e", name="et")
                nc.scalar.activation(e_t[:], pst[:], AF.Exp, scale=0.125)
                first = tp == 0
                last = tp == N_PAIR - 1
                for cc in range(2):
                    nc.tensor.matmul(
                        o_acc[cc][:],
                        vtg[:, it0 : it0 + 2, 128 * cc : 128 * (cc + 1)],
                        e_t[:],
                        start=first, stop=last, perf_mode=DR,
                    )
                nc.tensor.matmul(
                    den[:], ones_pair[:], e_t[:],
                    start=first, stop=last, perf_mode=DR,
                )

            def epilogue_a(jc, o_acc, den):
                # reciprocal chain first (it gates the rb matmul), then free
                # the PSUM accumulators
                den_sb = miscp.tile([1, JC], F32, tag="densb", bufs=2)
                nc.vector.tensor_copy(den_sb[:], den[0:1, :])
                rden = miscp.tile([1, JC], F32, tag="rden", bufs=2)
                nc.vector.reciprocal(rden[:], den_sb[:])
                rden_bf = miscp.tile([1, JC], BF16, tag="rdenbf", bufs=2)
                nc.vector.tensor_copy(rden_bf[:], rden[:])
                oc_sb = []
                for cc in range(2):
                    t = osbp.tile([128, JC], F32, tag=f"ocp{cc}", name=f"ocp{cc}")
                    nc.vector.tensor_copy(t[:], o_acc[cc][:])
                    oc_sb.append(t)
                return oc_sb, rden_bf

            def epilogue_b(jc, oc_sb, rden_bf):
                # broadcast 1/den across partitions via a DRAM bounce with a
                # stride-0 read: keeps the PE (and a PSUM bank) out of the
                # epilogue entirely, so chunk boundaries never stall the
                # matmul queue
                jsl = slice(JC * jc, JC * (jc + 1))
                rden_d = drp.tile([1, JC], BF16, tag="rdend")
                nc.sync.dma_start(rden_d[:], rden_bf[:])
                rb_sb = osbp.tile([128, JC], BF16, tag="rb")
                rden_bc = bass.AP(rden_d.tensor, rden_d.offset, [[0, 128], [1, JC]])
                nc.sync.dma_start(rb_sb[:], rden_bc)
                for cc in range(2):
                    o_n = osbp.tile([128, JC], F32, tag="on")
                    nc.vector.tensor_mul(o_n[:], oc_sb[cc][:], rb_sb[:])
                    res = osbp.tile([128, JC], F32, tag="res")
                    nc.vector.scalar_tensor_tensor(
                        res[:],
                        in0=o_n[:],
                        scalar=gbv[cc][:],
                        in1=x_sb[cc][:, jsl],
                        op0=mybir.AluOpType.add,
                        op1=mybir.AluOpType.add,
                    )
                    nc.sync.dma_start(
                        out_ext.ap()[128 * cc : 128 * (cc + 1), jsl], res[:]
                    )

            # ---- emission: prologue runs 2 chunks ahead of the j-chunk-0
            # pairs that consume it; epilogues split across the boundary ----
            def new_acc():
                o_acc = [
                    psacc.tile([128, JC], F32, tag=f"oacc{cc}", name=f"oacc{cc}")
                    for cc in range(2)
                ]
                den = pse.tile([32, JC], F32, tag="epil", name="den")
                return o_acc, den

            prologue(0)
            prologue(1)
            o_acc0, den0 = new_acc()
            for tp in range(4):
                attn_pair(0, tp, o_acc0, den0)
            prologue(2)
            for tp in range(4, 8):
                attn_pair(0, tp, o_acc0, den0)
            prologue(3)
            for tp in range(8, N_PAIR):
                attn_pair(0, tp, o_acc0, den0)
            carry = epilogue_a(0, o_acc0, den0)

            for jc in range(1, N_JC):
                o_acc, den = new_acc()
                for tp in range(5):
                    attn_pair(jc, tp, o_acc, den)
                epilogue_b(jc - 1, *carry)
                for tp in range(5, N_PAIR):
                    attn_pair(jc, tp, o_acc, den)
                carry = epilogue_a(jc, o_acc, den)
            epilogue_b(N_JC - 1, *carry)

    if split_waits:
        _split_multi_waits(nc)
    return nc


_NC_CACHE = None


def kernel(x, Wqkv, bqkv, gamma):
    global _NC_CACHE
    if _NC_CACHE is None:
        _NC_CACHE = build()
    nc = _NC_CACHE
    B = x.shape[0]
    assert B == N_CORES
    in_maps = []
    for i in range(B):
        in_maps.append(
            {
                "x": np.ascontiguousarray(x[i].reshape(C, HW), dtype=np.float32),
                "Wqkv": np.ascontiguousarray(Wqkv, dtype=np.float32),
                "bqkv": np.ascontiguousarray(np.asarray(bqkv).reshape(2 * M + C, 1), dtype=np.float32),
                "gamma": np.ascontiguousarray(np.asarray(gamma).reshape(1, 1), dtype=np.float32),
            }
        )
    res = run_bass_kernel_spmd(nc, in_maps, core_ids=list(range(N_CORES)))
    out = np.stack(
        [res.results[i]["out"].reshape(C, 64, 64) for i in range(N_CORES)]
    ).astype(np.float32)
    return out


# revision 5
# speedup vs baseline: 1.0623x; 1.0293x over previous
"""Trainium2 Bass kernel for nn_AttentionLayer (final, ~218 us vs 425 us baseline).

Data-parallel over batch: each of the 8 NeuronCores runs one image
[256, 64*64] end-to-end (QKV 1x1-conv, q^T k scores, softmax over the
query axis, v @ P, residual) with the small weights replicated.

Key design points (arrived at via neuron-profile trace iteration):
  - fp8(e4m3) DoubleRow matmuls for the QKV projection, the PV
    accumulation and the ones-row den reduction: 2 contraction k-tiles
    (256 rows) per 512-column pass, halving PE streaming work vs bf16.
    Scores stay bf16 as two concurrent K=64 row-group matmuls.
  - score PSUM tiles are [128, 2, 512] pairs: ONE exp per pair on the
    scalar engine (128 exps x ~1.11 us = the second bottleneck),
    writing fp8 in exactly the [K, 2, N] layout DoubleRow consumes.
  - softmax denominator via a DoubleRow ones-matmul; 1/den on DVE off
    the critical path; the partition broadcast of 1/den is a
    stride-0-read DMA through a DRAM bounce, keeping the PE (and a
    PSUM bank) out of the per-chunk epilogue entirely -- chunk
    boundaries never stall the matmul queue.
  - PSUM budget: score pairs 2x2 banks, o_acc 2, den 2 (dbl-buffered).
  - prologue (x DMA, relu+fp8 quantize on DVE, QKV, vtg quantize on
    ACT) runs in 1024-column steps software-pipelined 2 steps ahead of
    the j-chunk-0 attention pairs; weight/const DMAs issue from the
    Activation hwdge queue in parallel with x DMAs on Sync.
  - per-chunk epilogue split: PSUM-freeing copies + reciprocal at the
    chunk boundary, normalize/residual/DMA-out emitted 5 pairs into
    the next chunk.

Hardware context: the PE runs at an effective ~1.4 GHz here (power
throttling caps utilization ~25% of the time); steady-state is PE/ACT
co-limited at ~1.25/1.11 us per i-tile pair, boundary-gap free.
"""

import numpy as np

import bass_rust
import concourse.bass as bass
import concourse.tile as tile
from concourse import mybir
from concourse.bass_utils import run_bass_kernel_spmd

N_CORES = 8
C = 256
M = 64
HW = 4096
JC = 512
N_JC = HW // JC          # 8 j-chunks
N_IT = HW // 128         # 32 i-tiles
N_PAIR = N_IT // 2       # 16 i-tile pairs per j-chunk

F32 = mybir.dt.float32
BF16 = mybir.dt.bfloat16
F8 = mybir.dt.float8e4
DR = mybir.MatmulPerfMode.DoubleRow
AF = mybir.ActivationFunctionType


def _install_tile_drain_fix():
    def _drain_and_barrier(self, tick_clock, wait_clock):
        from concourse.tile import ScopedClock

        nc = self.nc
        probe = nc.sync.nop()
        wait_clock.add_sem_waits(
            probe.ins, ScopedClock({None: tick_clock.global_clock})
        )
        si = probe.ins.sync_info
        waits = list(si.on_wait) if si is not None else []
        probe.ins.sync_info = bass_rust.SyncInfo(on_wait=waits[:1], on_update=[])
        for w in waits[1:]:
            n = nc.sync.nop()
            n.ins.sync_info = bass_rust.SyncInfo(on_wait=[w], on_update=[])
        nc.sync.drain()
        nc.all_engine_barrier()
        assert self.sems is not None
        popped = nc._tile_sem_poison_stack.pop()
        assert popped is self._sem_poison
        nc.clear_and_free_semaphores(list(self.sems.allocated().values()))
        nc.all_engine_barrier()

    tile.TileContext._drain_and_barrier = _drain_and_barrier


def _split_multi_waits(nc):
    """walrus in this toolchain encodes at most one sync wait per
    instruction. Split any instruction carrying more onto single-wait
    NOPs inserted immediately before it on the same engine (in-order
    engines make this semantics-preserving)."""
    ctr = [0]

    def mk_nop(engine, wait):
        ctr[0] += 1
        n = mybir.InstNoOp(name=f"I-wsplit{ctr[0]}", ins=[], outs=[])
        n.engine = engine
        n.sync_info = bass_rust.SyncInfo(on_wait=[wait], on_update=[])
        return n

    for f in nc.m.functions:
        for bb in f.blocks:
            out = []
            changed = False
            for inst in bb.instructions:
                si = inst.sync_info
                waits = list(si.on_wait) if si is not None else []
                if len(waits) > 1:
                    for w in waits[:-1]:
                        out.append(mk_nop(inst.engine, w))
                    inst.sync_info = bass_rust.SyncInfo(
                        on_wait=[waits[-1]], on_update=list(si.on_update)
                    )
                    changed = True
                out.append(inst)
            if changed:
                bb.instructions = out


def build(split_waits=True):
    _install_tile_drain_fix()
    nc = bass.Bass("TRN2", target_bir_lowering=False, debug=False)

    x_ext = nc.declare_dram_parameter("x", [C, HW], F32, isOutput=False)
    w_ext = nc.declare_dram_parameter("Wqkv", [2 * M + C, C], F32, isOutput=False)
    b_ext = nc.declare_dram_parameter("bqkv", [2 * M + C, 1], F32, isOutput=False)
    g_ext = nc.declare_dram_parameter("gamma", [1, 1], F32, isOutput=False)
    out_ext = nc.declare_dram_parameter("out", [C, HW], F32, isOutput=True)

    ident_dram = nc.inline_tensor(np.eye(128, dtype=np.float32), "ident128")
    onesr_dram = nc.inline_tensor(np.ones((1, 128), dtype=np.float32), "onesrow")

    with tile.TileContext(nc) as tc:
        with (
            tc.tile_pool(name="const", bufs=1) as constp,
            tc.tile_pool(name="wld", bufs=1) as wldp,
            tc.tile_pool(name="wt", bufs=1) as wtp,
            tc.tile_pool(name="xin", bufs=1) as xp,
            tc.tile_pool(name="xr", bufs=1) as xrp,
            tc.tile_pool(name="qk", bufs=1) as qkp,
            tc.tile_pool(name="vt", bufs=1) as vtp,
            tc.tile_pool(name="e", bufs=6) as ep,
            tc.tile_pool(name="osb", bufs=2) as osbp,
            tc.tile_pool(name="misc", bufs=1) as miscp,
            # PSUM budget (16KB/partition = 8 banks):
            #   beta pairs [128,2,512]f32 x2 bufs = 8KB
            #   o_acc 2 x [128,512]f32          = 4KB
            #   epil (den [1,512] / rb [128,512]) x2 bufs = 4KB
            tc.tile_pool(name="ps_b", bufs=2, space="PSUM") as psb,
            tc.tile_pool(name="ps_acc", bufs=1, space="PSUM") as psacc,
            tc.tile_pool(name="ps_e", bufs=2, space="PSUM") as pse,
            tc.tile_pool(name="drscratch", bufs=2, space="DRAM") as drp,
        ):
            # ---- constants ----
            ident = constp.tile([128, 128], F32)
            nc.scalar.dma_start(ident[:], ident_dram.ap()[:, :])
            ones_row = constp.tile([1, 128], F32)
            nc.scalar.dma_start(ones_row[:], onesr_dram.ap()[:, :])
            ones_row_bf = constp.tile([1, 128], BF16)
            nc.vector.tensor_copy(ones_row_bf[:], ones_row[:])
            ones_pair = constp.tile([128, 2, 32], F8)
            nc.vector.memset(ones_pair[:], 1.0)

            # ---- load W rows, biases, gamma ----
            w_rows = []
            for oc in range(3):
                wt_ = wldp.tile([128, C], F32, tag=f"wrows{oc}", name=f"wrows{oc}")
                nc.scalar.dma_start(wt_[:], w_ext.ap()[128 * oc : 128 * (oc + 1), :])
                w_rows.append(wt_)
            bias_qq = miscp.tile([128, 1], F32, tag="bqq")
            nc.sync.dma_start(bias_qq[0:64, :], b_ext.ap()[0:64, :])
            nc.sync.dma_start(bias_qq[64:128, :], b_ext.ap()[0:64, :])
            bias_kk = miscp.tile([128, 1], F32, tag="bkk")
            nc.sync.dma_start(bias_kk[0:64, :], b_ext.ap()[64:128, :])
            nc.sync.dma_start(bias_kk[64:128, :], b_ext.ap()[64:128, :])
            bias_v = []
            for cc in range(2):
                bv = miscp.tile([128, 1], F32, tag=f"bv{cc}", name=f"bv{cc}")
                nc.sync.dma_start(
                    bv[:], b_ext.ap()[128 + 128 * cc : 128 + 128 * (cc + 1), :]
                )
                bias_v.append(bv)
            gamma_sb = miscp.tile([1, 1], F32, tag="gam")
            nc.sync.dma_start(gamma_sb[:], g_ext.ap()[:, :])

            gamma_bf = miscp.tile([1, 1], BF16, tag="gambf")
            nc.vector.tensor_copy(gamma_bf[:], gamma_sb[:])
            g_ps = psb.tile([128, 1], F32, tag="beta", name="g_ps")
            nc.tensor.matmul(g_ps[:], ones_row_bf[:], gamma_bf[:], start=True, stop=True)
            gamma_bc = miscp.tile([128, 1], F32, tag="gbc_sb")
            nc.vector.tensor_copy(gamma_bc[:], g_ps[:])
            gbv = []
            for cc in range(2):
                t = miscp.tile([128, 1], F32, tag=f"gbv{cc}", name=f"gbv{cc}")
                nc.vector.tensor_mul(t[:], bias_v[cc][:], gamma_bc[:])
                gbv.append(t)

            # ---- transpose W via PE, pack into fp8 DoubleRow layouts ----
            # wq_pack/wk_pack: [c%128, c-tile, m-dup] ; wv_pack: [c%128, c-tile, ch]
            wq_pack = wtp.tile([128, 2, 128], F8, tag="wq")
            wk_pack = wtp.tile([128, 2, 128], F8, tag="wk")
            wv_pack = wtp.tile([128, 2, 256], F8, tag="wv")
            for cc in range(2):
                ps = psb.tile([128, 128], F32, tag="beta", name="wtp_ps")
                nc.tensor.transpose(ps[:], w_rows[0][:, 128 * cc : 128 * (cc + 1)], ident[:])
                nc.vector.tensor_copy(wq_pack[:, cc, 0:64], ps[:, 0:64])
                nc.vector.tensor_copy(wq_pack[:, cc, 64:128], ps[:, 0:64])
                nc.vector.tensor_copy(wk_pack[:, cc, 0:64], ps[:, 64:128])
                nc.vector.tensor_copy(wk_pack[:, cc, 64:128], ps[:, 64:128])
            for oc in range(2):
                for cc in range(2):
                    ps = psb.tile([128, 128], F32, tag="beta", name="wtp_ps")
                    nc.tensor.transpose(
                        ps[:], w_rows[1 + oc][:, 128 * cc : 128 * (cc + 1)], ident[:]
                    )
                    nc.vector.tensor_copy(
                        wv_pack[:, cc, 128 * oc : 128 * (oc + 1)], ps[:]
                    )

            # ---- persistent SBUF tensors ----
            x_sb = [
                xp.tile([128, HW], F32, tag=f"x{cc}", name=f"xchunk{cc}")
                for cc in range(2)
            ]
            xr_pack = xrp.tile([128, 2, HW], F8, tag="xr")
            qq_sb = qkp.tile([128, HW], BF16, tag="qq")
            kk_sb = qkp.tile([128, HW], BF16, tag="kk")
            vtg = vtp.tile([128, N_IT, 256], F8, tag="vtg")

            PC = 2 * JC  # 1024-wide prologue steps

            def prologue(k):
                sl = slice(PC * k, PC * (k + 1))
                for cc in range(2):
                    nc.sync.dma_start(
                        x_sb[cc][:, sl], x_ext.ap()[128 * cc : 128 * (cc + 1), sl]
                    )
                    # relu + fp8 quantize on DVE (ACT is the exp bottleneck)
                    nc.vector.tensor_scalar_max(
                        xr_pack[:, cc, sl], x_sb[cc][:, sl], 0.0
                    )
                # k first (j-chunk 0 needs kk[:, 0:512] earliest)
                # one PSUM alloc per projection, two bank-aligned N=512
                # matmuls (a matmul output cannot span PSUM banks)
                kps = psb.tile([128, PC], F32, tag="beta", name="kps")
                for h in range(2):
                    hsl = slice(PC * k + JC * h, PC * k + JC * (h + 1))
                    nc.tensor.matmul(
                        kps[:, JC * h : JC * (h + 1)], wk_pack[:, :, :],
                        xr_pack[:, :, hsl], start=True, stop=True, perf_mode=DR,
                    )
                nc.vector.tensor_scalar_add(kk_sb[:, sl], kps[:], bias_kk[:])
                qps = psb.tile([128, PC], F32, tag="beta", name="qps")
                for h in range(2):
                    hsl = slice(PC * k + JC * h, PC * k + JC * (h + 1))
                    nc.tensor.matmul(
                        qps[:, JC * h : JC * (h + 1)], wq_pack[:, :, :],
                        xr_pack[:, :, hsl], start=True, stop=True, perf_mode=DR,
                    )
                nc.vector.tensor_scalar_add(qq_sb[:, sl], qps[:], bias_qq[:])
                # v for the 8 i-tiles of this step, batched 4 per PSUM alloc
                # (2 i-tiles share a 2KB bank: start on the first, fresh-byte
                # accumulate on the second)
                for q in range(2):
                    pq = psb.tile([128, 4, 256], F32, tag="beta", name="pq")
                    for t4 in range(4):
                        tt = 8 * k + 4 * q + t4
                        nc.tensor.matmul(
                            pq[:, t4, :],
                            xr_pack[:, :, 128 * tt : 128 * (tt + 1)],
                            wv_pack[:, :, :],
                            start=(t4 % 2 == 0), stop=(t4 % 2 == 1),
                            perf_mode=DR, skip_group_check=True,
                        )
                    nc.scalar.mul(
                        vtg[:, 8 * k + 4 * q : 8 * k + 4 * q + 4, :],
                        pq[:], gamma_bc[:],
                    )

            def attn_pair(jc, tp, o_acc, den):
                jsl = slice(JC * jc, JC * (jc + 1))
                it0, it1 = 2 * tp, 2 * tp + 1
                pst = psb.tile([128, 2, JC], F32, tag="beta", name="pst")
                nc.tensor.matmul(
                    pst[:, 0, :],
                    qq_sb[0:64, 128 * it0 : 128 * (it0 + 1)],
                    kk_sb[0:64, jsl],
                    start=True, stop=True,
                )
                nc.tensor.matmul(
                    pst[:, 1, :],
                    qq_sb[64:128, 128 * it1 : 128 * (it1 + 1)],
                    kk_sb[64:128, jsl],
                    start=True, stop=True,
                )
                e_t = ep.tile([128, 2, JC], F8, tag="e", name="et")
                nc.scalar.activation(e_t[:], pst[:], AF.Exp, scale=0.125)
                first = tp == 0
                last = tp == N_PAIR - 1
                for cc in range(2):
                    nc.tensor.matmul(
                        o_acc[cc][:],
                        vtg[:, it0 : it0 + 2, 128 * cc : 128 * (cc + 1)],
                        e_t[:],
                        start=first, stop=last, perf_mode=DR,
                    )
                nc.tensor.matmul(
                    den[:], ones_pair[:], e_t[:],
                    start=first, stop=last, perf_mode=DR,
                )

            def epilogue_a(jc, o_acc, den):
                # reciprocal chain first (it gates the rb matmul), then free
                # the PSUM accumulators
                den_sb = miscp.tile([1, JC], F32, tag="densb", bufs=2)
                nc.vector.tensor_copy(den_sb[:], den[0:1, :])
                rden = miscp.tile([1, JC], F32, tag="rden", bufs=2)
                nc.vector.reciprocal(rden[:], den_sb[:])
                rden_bf = miscp.tile([1, JC], BF16, tag="rdenbf", bufs=2)
                nc.vector.tensor_copy(rden_bf[:], rden[:])
                oc_sb = []
                for cc in range(2):
                    t = osbp.tile([128, JC], F32, tag=f"ocp{cc}", name=f"ocp{cc}")
                    nc.vector.tensor_copy(t[:], o_acc[cc][:])
                    oc_sb.append(t)
                return oc_sb, rden_bf

            def epilogue_b(jc, oc_sb, rden_bf):
                # broadcast 1/den across partitions via a DRAM bounce with a
                # stride-0 read: keeps the PE (and a PSUM bank) out of the
                # epilogue entirely, so chunk boundaries never stall the
                # matmul queue
                jsl = slice(JC * jc, JC * (jc + 1))
                rden_d = drp.tile([1, JC], BF16, tag="rdend")
                nc.sync.dma_start(rden_d[:], rden_bf[:])
                rb_sb = osbp.tile([128, JC], BF16, tag="rb")
                rden_bc = bass.AP(rden_d.tensor, rden_d.offset, [[0, 128], [1, JC]])
                nc.sync.dma_start(rb_sb[:], rden_bc)
                for cc in range(2):
                    o_n = osbp.tile([128, JC], F32, tag="on")
                    nc.vector.tensor_mul(o_n[:], oc_sb[cc][:], rb_sb[:])
                    res = osbp.tile([128, JC], F32, tag="res")
                    nc.vector.scalar_tensor_tensor(
                        res[:],
                        in0=o_n[:],
                        scalar=gbv[cc][:],
                        in1=x_sb[cc][:, jsl],
                        op0=mybir.AluOpType.add,
                        op1=mybir.AluOpType.add,
                    )
                    nc.sync.dma_start(
                        out_ext.ap()[128 * cc : 128 * (cc + 1), jsl], res[:]
                    )

            # ---- emission: prologue runs 2 chunks ahead of the j-chunk-0
            # pairs that consume it; epilogues split across the boundary ----
            def new_acc():
                o_acc = [
                    psacc.tile([128, JC], F32, tag=f"oacc{cc}", name=f"oacc{cc}")
                    for cc in range(2)
                ]
                den = pse.tile([32, JC], F32, tag="epil", name="den")
                return o_acc, den

            prologue(0)
            prologue(1)
            o_acc0, den0 = new_acc()
            for tp in range(4):
                attn_pair(0, tp, o_acc0, den0)
            prologue(2)
            for tp in range(4, 8):
                attn_pair(0, tp, o_acc0, den0)
            prologue(3)
            for tp in range(8, N_PAIR):
                attn_pair(0, tp, o_acc0, den0)
            carry = epilogue_a(0, o_acc0, den0)

            for jc in range(1, N_JC):
                o_acc, den = new_acc()
                for tp in range(5):
                    attn_pair(jc, tp, o_acc, den)
                epilogue_b(jc - 1, *carry)
                for tp in range(5, N_PAIR):
                    attn_pair(jc, tp, o_acc, den)
                carry = epilogue_a(jc, o_acc, den)
            epilogue_b(N_JC - 1, *carry)

    if split_waits:
        _split_multi_waits(nc)
    return nc


_NC_CACHE = None


def kernel(x, Wqkv, bqkv, gamma):
    global _NC_CACHE
    if _NC_CACHE is None:
        _NC_CACHE = build()
    nc = _NC_CACHE
    B = x.shape[0]
    assert B == N_CORES
    in_maps = []
    for i in range(B):
        in_maps.append(
            {
                "x": np.ascontiguousarray(x[i].reshape(C, HW), dtype=np.float32),
                "Wqkv": np.ascontiguousarray(Wqkv, dtype=np.float32),
                "bqkv": np.ascontiguousarray(np.asarray(bqkv).reshape(2 * M + C, 1), dtype=np.float32),
                "gamma": np.ascontiguousarray(np.asarray(gamma).reshape(1, 1), dtype=np.float32),
            }
        )
    res = run_bass_kernel_spmd(nc, in_maps, core_ids=list(range(N_CORES)))
    out = np.stack(
        [res.results[i]["out"].reshape(C, 64, 64) for i in range(N_CORES)]
    ).astype(np.float32)
    return out


# revision 6
# speedup vs baseline: 1.0671x; 1.0045x over previous
"""Trainium2 Bass kernel for nn_AttentionLayer (final, ~211 us vs 425 us baseline).

Data-parallel over batch: each of the 8 NeuronCores runs one image
[256, 64*64] end-to-end (QKV 1x1-conv, q^T k scores, softmax over the
query axis, v @ P, residual) with the small weights replicated.

Key design points (arrived at via neuron-profile trace iteration):
  - fp8(e4m3) DoubleRow matmuls for the QKV projection, the PV
    accumulation and the ones-row den reduction: 2 contraction k-tiles
    (256 rows) per 512-column pass, halving PE streaming work vs bf16.
    Scores stay bf16 as two concurrent K=64 row-group matmuls.
  - score PSUM tiles are [128, 2, 512] pairs: ONE exp per pair on the
    scalar engine (128 exps x ~1.11 us = the second bottleneck),
    writing fp8 in exactly the [K, 2, N] layout DoubleRow consumes.
  - softmax denominator via a DoubleRow ones-matmul; 1/den on DVE off
    the critical path; its partition broadcast is a stride-0-read DMA
    through a DRAM bounce, keeping the PE (and a PSUM bank) out of the
    per-chunk epilogue -- chunk boundaries never stall the matmul
    queue.  The last chunk (idle PE, DMA latency = serial tail) uses
    the rank-1 matmul broadcast instead.
  - PSUM budget: score pairs 2x2 banks, o_acc 2, den 2 (dbl-buffered).
  - prologue (x DMA, relu+fp8 quantize on DVE, QKV, vtg quantize on
    ACT) runs in 1024-column steps interleaved one step ahead of the
    j-chunk-0 attention pairs that consume them, so the exp stream
    starts ~15 us in; weight/const DMAs issue from the Activation
    hwdge queue in parallel with x DMAs on Sync.
  - per-chunk epilogue split: PSUM-freeing copies + reciprocal at the
    chunk boundary, normalize/residual/DMA-out emitted 5 pairs into
    the next chunk.

Hardware context: the PE runs at an effective ~1.4 GHz here (power
throttling caps utilization ~25% of the time; runs land in a fast
~211-225 us state or an occasional throttled ~258 us state); GPSIMD
cannot access PSUM; steady-state is PE/ACT co-limited at ~1.25/1.11 us
per i-tile pair, boundary-gap free.
"""

import numpy as np

import bass_rust
import concourse.bass as bass
import concourse.tile as tile
from concourse import mybir
from concourse.bass_utils import run_bass_kernel_spmd

N_CORES = 8
C = 256
M = 64
HW = 4096
JC = 512
N_JC = HW // JC          # 8 j-chunks
N_IT = HW // 128         # 32 i-tiles
N_PAIR = N_IT // 2       # 16 i-tile pairs per j-chunk

F32 = mybir.dt.float32
BF16 = mybir.dt.bfloat16
F8 = mybir.dt.float8e4
DR = mybir.MatmulPerfMode.DoubleRow
AF = mybir.ActivationFunctionType


def _install_tile_drain_fix():
    def _drain_and_barrier(self, tick_clock, wait_clock):
        from concourse.tile import ScopedClock

        nc = self.nc
        probe = nc.sync.nop()
        wait_clock.add_sem_waits(
            probe.ins, ScopedClock({None: tick_clock.global_clock})
        )
        si = probe.ins.sync_info
        waits = list(si.on_wait) if si is not None else []
        probe.ins.sync_info = bass_rust.SyncInfo(on_wait=waits[:1], on_update=[])
        for w in waits[1:]:
            n = nc.sync.nop()
            n.ins.sync_info = bass_rust.SyncInfo(on_wait=[w], on_update=[])
        nc.sync.drain()
        nc.all_engine_barrier()
        assert self.sems is not None
        popped = nc._tile_sem_poison_stack.pop()
        assert popped is self._sem_poison
        nc.clear_and_free_semaphores(list(self.sems.allocated().values()))
        nc.all_engine_barrier()

    tile.TileContext._drain_and_barrier = _drain_and_barrier


def _split_multi_waits(nc):
    """walrus in this toolchain encodes at most one sync wait per
    instruction. Split any instruction carrying more onto single-wait
    NOPs inserted immediately before it on the same engine (in-order
    engines make this semantics-preserving)."""
    ctr = [0]

    def mk_nop(engine, wait):
        ctr[0] += 1
        n = mybir.InstNoOp(name=f"I-wsplit{ctr[0]}", ins=[], outs=[])
        n.engine = engine
        n.sync_info = bass_rust.SyncInfo(on_wait=[wait], on_update=[])
        return n

    for f in nc.m.functions:
        for bb in f.blocks:
            out = []
            changed = False
            for inst in bb.instructions:
                si = inst.sync_info
                waits = list(si.on_wait) if si is not None else []
                if len(waits) > 1:
                    for w in waits[:-1]:
                        out.append(mk_nop(inst.engine, w))
                    inst.sync_info = bass_rust.SyncInfo(
                        on_wait=[waits[-1]], on_update=list(si.on_update)
                    )
                    changed = True
                out.append(inst)
            if changed:
                bb.instructions = out


def build(split_waits=True):
    _install_tile_drain_fix()
    nc = bass.Bass("TRN2", target_bir_lowering=False, debug=False)

    x_ext = nc.declare_dram_parameter("x", [C, HW], F32, isOutput=False)
    w_ext = nc.declare_dram_parameter("Wqkv", [2 * M + C, C], F32, isOutput=False)
    b_ext = nc.declare_dram_parameter("bqkv", [2 * M + C, 1], F32, isOutput=False)
    g_ext = nc.declare_dram_parameter("gamma", [1, 1], F32, isOutput=False)
    out_ext = nc.declare_dram_parameter("out", [C, HW], F32, isOutput=True)

    ident_dram = nc.inline_tensor(np.eye(128, dtype=np.float32), "ident128")
    onesr_dram = nc.inline_tensor(np.ones((1, 128), dtype=np.float32), "onesrow")

    with tile.TileContext(nc) as tc:
        with (
            tc.tile_pool(name="const", bufs=1) as constp,
            tc.tile_pool(name="wld", bufs=1) as wldp,
            tc.tile_pool(name="wt", bufs=1) as wtp,
            tc.tile_pool(name="xin", bufs=1) as xp,
            tc.tile_pool(name="xr", bufs=1) as xrp,
            tc.tile_pool(name="qk", bufs=1) as qkp,
            tc.tile_pool(name="vt", bufs=1) as vtp,
            tc.tile_pool(name="e", bufs=6) as ep,
            tc.tile_pool(name="osb", bufs=2) as osbp,
            tc.tile_pool(name="misc", bufs=1) as miscp,
            # PSUM budget (16KB/partition = 8 banks):
            #   beta pairs [128,2,512]f32 x2 bufs = 8KB
            #   o_acc 2 x [128,512]f32          = 4KB
            #   epil (den [1,512] / rb [128,512]) x2 bufs = 4KB
            tc.tile_pool(name="ps_b", bufs=2, space="PSUM") as psb,
            tc.tile_pool(name="ps_acc", bufs=1, space="PSUM") as psacc,
            tc.tile_pool(name="ps_e", bufs=2, space="PSUM") as pse,
            tc.tile_pool(name="drscratch", bufs=2, space="DRAM") as drp,
        ):
            # ---- constants ----
            ident = constp.tile([128, 128], F32)
            nc.scalar.dma_start(ident[:], ident_dram.ap()[:, :])
            ones_row = constp.tile([1, 128], F32)
            nc.scalar.dma_start(ones_row[:], onesr_dram.ap()[:, :])
            ones_row_bf = constp.tile([1, 128], BF16)
            nc.vector.tensor_copy(ones_row_bf[:], ones_row[:])
            ones_pair = constp.tile([128, 2, 32], F8)
            nc.vector.memset(ones_pair[:], 1.0)

            # ---- load W rows, biases, gamma ----
            w_rows = []
            for oc in range(3):
                wt_ = wldp.tile([128, C], F32, tag=f"wrows{oc}", name=f"wrows{oc}")
                nc.scalar.dma_start(wt_[:], w_ext.ap()[128 * oc : 128 * (oc + 1), :])
                w_rows.append(wt_)
            bias_qq = miscp.tile([128, 1], F32, tag="bqq")
            nc.sync.dma_start(bias_qq[0:64, :], b_ext.ap()[0:64, :])
            nc.sync.dma_start(bias_qq[64:128, :], b_ext.ap()[0:64, :])
            bias_kk = miscp.tile([128, 1], F32, tag="bkk")
            nc.sync.dma_start(bias_kk[0:64, :], b_ext.ap()[64:128, :])
            nc.sync.dma_start(bias_kk[64:128, :], b_ext.ap()[64:128, :])
            bias_v = []
            for cc in range(2):
                bv = miscp.tile([128, 1], F32, tag=f"bv{cc}", name=f"bv{cc}")
                nc.sync.dma_start(
                    bv[:], b_ext.ap()[128 + 128 * cc : 128 + 128 * (cc + 1), :]
                )
                bias_v.append(bv)
            gamma_sb = miscp.tile([1, 1], F32, tag="gam")
            nc.sync.dma_start(gamma_sb[:], g_ext.ap()[:, :])

            gamma_bf = miscp.tile([1, 1], BF16, tag="gambf")
            nc.vector.tensor_copy(gamma_bf[:], gamma_sb[:])
            g_ps = psb.tile([128, 1], F32, tag="beta", name="g_ps")
            nc.tensor.matmul(g_ps[:], ones_row_bf[:], gamma_bf[:], start=True, stop=True)
            gamma_bc = miscp.tile([128, 1], F32, tag="gbc_sb")
            nc.vector.tensor_copy(gamma_bc[:], g_ps[:])
            gbv = []
            for cc in range(2):
                t = miscp.tile([128, 1], F32, tag=f"gbv{cc}", name=f"gbv{cc}")
                nc.vector.tensor_mul(t[:], bias_v[cc][:], gamma_bc[:])
                gbv.append(t)

            # ---- transpose W via PE, pack into fp8 DoubleRow layouts ----
            # wq_pack/wk_pack: [c%128, c-tile, m-dup] ; wv_pack: [c%128, c-tile, ch]
            wq_pack = wtp.tile([128, 2, 128], F8, tag="wq")
            wk_pack = wtp.tile([128, 2, 128], F8, tag="wk")
            wv_pack = wtp.tile([128, 2, 256], F8, tag="wv")
            for cc in range(2):
                ps = psb.tile([128, 128], F32, tag="beta", name="wtp_ps")
                nc.tensor.transpose(ps[:], w_rows[0][:, 128 * cc : 128 * (cc + 1)], ident[:])
                nc.vector.tensor_copy(wq_pack[:, cc, 0:64], ps[:, 0:64])
                nc.vector.tensor_copy(wq_pack[:, cc, 64:128], ps[:, 0:64])
                nc.vector.tensor_copy(wk_pack[:, cc, 0:64], ps[:, 64:128])
                nc.vector.tensor_copy(wk_pack[:, cc, 64:128], ps[:, 64:128])
            for oc in range(2):
                for cc in range(2):
                    ps = psb.tile([128, 128], F32, tag="beta", name="wtp_ps")
                    nc.tensor.transpose(
                        ps[:], w_rows[1 + oc][:, 128 * cc : 128 * (cc + 1)], ident[:]
                    )
                    nc.vector.tensor_copy(
                        wv_pack[:, cc, 128 * oc : 128 * (oc + 1)], ps[:]
                    )

            # ---- persistent SBUF tensors ----
            x_sb = [
                xp.tile([128, HW], F32, tag=f"x{cc}", name=f"xchunk{cc}")
                for cc in range(2)
            ]
            xr_pack = xrp.tile([128, 2, HW], F8, tag="xr")
            qq_sb = qkp.tile([128, HW], BF16, tag="qq")
            kk_sb = qkp.tile([128, HW], BF16, tag="kk")
            vtg = vtp.tile([128, N_IT, 256], F8, tag="vtg")

            PC = 2 * JC  # 1024-wide prologue steps

            def prologue(k):
                sl = slice(PC * k, PC * (k + 1))
                for cc in range(2):
                    nc.sync.dma_start(
                        x_sb[cc][:, sl], x_ext.ap()[128 * cc : 128 * (cc + 1), sl]
                    )
                    # relu + fp8 quantize on DVE (ACT is the exp bottleneck)
                    nc.vector.tensor_scalar_max(
                        xr_pack[:, cc, sl], x_sb[cc][:, sl], 0.0
                    )
                # k first (j-chunk 0 needs kk[:, 0:512] earliest)
                # one PSUM alloc per projection, two bank-aligned N=512
                # matmuls (a matmul output cannot span PSUM banks)
                kps = psb.tile([128, PC], F32, tag="beta", name="kps")
                for h in range(2):
                    hsl = slice(PC * k + JC * h, PC * k + JC * (h + 1))
                    nc.tensor.matmul(
                        kps[:, JC * h : JC * (h + 1)], wk_pack[:, :, :],
                        xr_pack[:, :, hsl], start=True, stop=True, perf_mode=DR,
                    )
                nc.vector.tensor_scalar_add(kk_sb[:, sl], kps[:], bias_kk[:])
                qps = psb.tile([128, PC], F32, tag="beta", name="qps")
                for h in range(2):
                    hsl = slice(PC * k + JC * h, PC * k + JC * (h + 1))
                    nc.tensor.matmul(
                        qps[:, JC * h : JC * (h + 1)], wq_pack[:, :, :],
                        xr_pack[:, :, hsl], start=True, stop=True, perf_mode=DR,
                    )
                nc.vector.tensor_scalar_add(qq_sb[:, sl], qps[:], bias_qq[:])
                # v for the 8 i-tiles of this step, batched 4 per PSUM alloc
                # (2 i-tiles share a 2KB bank: start on the first, fresh-byte
                # accumulate on the second)
                for q in range(2):
                    pq = psb.tile([128, 4, 256], F32, tag="beta", name="pq")
                    for t4 in range(4):
                        tt = 8 * k + 4 * q + t4
                        nc.tensor.matmul(
                            pq[:, t4, :],
                            xr_pack[:, :, 128 * tt : 128 * (tt + 1)],
                            wv_pack[:, :, :],
                            start=(t4 % 2 == 0), stop=(t4 % 2 == 1),
                            perf_mode=DR, skip_group_check=True,
                        )
                    nc.scalar.mul(
                        vtg[:, 8 * k + 4 * q : 8 * k + 4 * q + 4, :],
                        pq[:], gamma_bc[:],
                    )

            def attn_pair(jc, tp, o_acc, den):
                jsl = slice(JC * jc, JC * (jc + 1))
                it0, it1 = 2 * tp, 2 * tp + 1
                pst = psb.tile([128, 2, JC], F32, tag="beta", name="pst")
                nc.tensor.matmul(
                    pst[:, 0, :],
                    qq_sb[0:64, 128 * it0 : 128 * (it0 + 1)],
                    kk_sb[0:64, jsl],
                    start=True, stop=True,
                )
                nc.tensor.matmul(
                    pst[:, 1, :],
                    qq_sb[64:128, 128 * it1 : 128 * (it1 + 1)],
                    kk_sb[64:128, jsl],
                    start=True, stop=True,
                )
                e_t = ep.tile([128, 2, JC], F8, tag="e", name="et")
                nc.scalar.activation(e_t[:], pst[:], AF.Exp, scale=0.125)
                first = tp == 0
                last = tp == N_PAIR - 1
                for cc in range(2):
                    nc.tensor.matmul(
                        o_acc[cc][:],
                        vtg[:, it0 : it0 + 2, 128 * cc : 128 * (cc + 1)],
                        e_t[:],
                        start=first, stop=last, perf_mode=DR,
                    )
                nc.tensor.matmul(
                    den[:], ones_pair[:], e_t[:],
                    start=first, stop=last, perf_mode=DR,
                )

            def epilogue_a(jc, o_acc, den):
                # reciprocal chain first (it gates the rb matmul), then free
                # the PSUM accumulators
                den_sb = miscp.tile([1, JC], F32, tag="densb", bufs=2)
                nc.vector.tensor_copy(den_sb[:], den[0:1, :])
                rden = miscp.tile([1, JC], F32, tag="rden", bufs=2)
                nc.vector.reciprocal(rden[:], den_sb[:])
                rden_bf = miscp.tile([1, JC], BF16, tag="rdenbf", bufs=2)
                nc.vector.tensor_copy(rden_bf[:], rden[:])
                oc_sb = []
                for cc in range(2):
                    t = osbp.tile([128, JC], F32, tag=f"ocp{cc}", name=f"ocp{cc}")
                    nc.vector.tensor_copy(t[:], o_acc[cc][:])
                    oc_sb.append(t)
                return oc_sb, rden_bf

            def epilogue_b(jc, oc_sb, rden_bf, last=False):
                # broadcast 1/den across partitions via a DRAM bounce with a
                # stride-0 read: keeps the PE (and a PSUM bank) out of the
                # epilogue entirely, so chunk boundaries never stall the
                # matmul queue.  For the last chunk the PE is idle and the
                # DMA latency is pure serial tail: use the matmul broadcast.
                jsl = slice(JC * jc, JC * (jc + 1))
                if last:
                    rb_ps = pse.tile([128, JC], F32, tag="epil", name="rb_l")
                    nc.tensor.matmul(
                        rb_ps[:], ones_row_bf[:], rden_bf[:], start=True, stop=True
                    )
                    rb_any = rb_ps
                else:
                    rden_d = drp.tile([1, JC], BF16, tag="rdend")
                    nc.sync.dma_start(rden_d[:], rden_bf[:])
                    rb_sb = osbp.tile([128, JC], BF16, tag="rb")
                    rden_bc = bass.AP(rden_d.tensor, rden_d.offset, [[0, 128], [1, JC]])
                    nc.sync.dma_start(rb_sb[:], rden_bc)
                    rb_any = rb_sb
                for cc in range(2):
                    o_n = osbp.tile([128, JC], F32, tag="on")
                    nc.vector.tensor_mul(o_n[:], oc_sb[cc][:], rb_any[:])
                    res = osbp.tile([128, JC], F32, tag="res")
                    nc.vector.scalar_tensor_tensor(
                        res[:],
                        in0=o_n[:],
                        scalar=gbv[cc][:],
                        in1=x_sb[cc][:, jsl],
                        op0=mybir.AluOpType.add,
                        op1=mybir.AluOpType.add,
                    )
                    nc.sync.dma_start(
                        out_ext.ap()[128 * cc : 128 * (cc + 1), jsl], res[:]
                    )

            # ---- emission: prologue runs 2 chunks ahead of the j-chunk-0
            # pairs that consume it; epilogues split across the boundary ----
            def new_acc():
                o_acc = [
                    psacc.tile([128, JC], F32, tag=f"oacc{cc}", name=f"oacc{cc}")
                    for cc in range(2)
                ]
                den = pse.tile([32, JC], F32, tag="epil", name="den")
                return o_acc, den

            prologue(0)
            o_acc0, den0 = new_acc()
            for tp in range(4):
                attn_pair(0, tp, o_acc0, den0)
            prologue(1)
            for tp in range(4, 8):
                attn_pair(0, tp, o_acc0, den0)
            prologue(2)
            for tp in range(8, 12):
                attn_pair(0, tp, o_acc0, den0)
            prologue(3)
            for tp in range(12, N_PAIR):
                attn_pair(0, tp, o_acc0, den0)
            carry = epilogue_a(0, o_acc0, den0)

            for jc in range(1, N_JC):
                o_acc, den = new_acc()
                for tp in range(5):
                    attn_pair(jc, tp, o_acc, den)
                epilogue_b(jc - 1, *carry)
                for tp in range(5, N_PAIR):
                    attn_pair(jc, tp, o_acc, den)
                carry = epilogue_a(jc, o_acc, den)
            epilogue_b(N_JC - 1, *carry, last=True)

    if split_waits:
        _split_multi_waits(nc)
    return nc


_NC_CACHE = None


def kernel(x, Wqkv, bqkv, gamma):
    global _NC_CACHE
    if _NC_CACHE is None:
        _NC_CACHE = build()
    nc = _NC_CACHE
    B = x.shape[0]
    assert B == N_CORES
    in_maps = []
    for i in range(B):
        in_maps.append(
            {
                "x": np.ascontiguousarray(x[i].reshape(C, HW), dtype=np.float32),
                "Wqkv": np.ascontiguousarray(Wqkv, dtype=np.float32),
                "bqkv": np.ascontiguousarray(np.asarray(bqkv).reshape(2 * M + C, 1), dtype=np.float32),
                "gamma": np.ascontiguousarray(np.asarray(gamma).reshape(1, 1), dtype=np.float32),
            }
        )
    res = run_bass_kernel_spmd(nc, in_maps, core_ids=list(range(N_CORES)))
    out = np.stack(
        [res.results[i]["out"].reshape(C, 64, 64) for i in range(N_CORES)]
    ).astype(np.float32)
    return out
